# revision 1
# baseline (speedup 1.0000x reference)
"""Trainium2 Bass kernel for a dense transformer block (PreNorm attn + MLP).

Full inputs: x [8, 1024, 768] f32 + LN/attn/MLP weights.
Sharding: pure data-parallel — batch 8 across 8 NeuronCores, no collectives.

Per-core design (tokens n=1024, d=768, heads=12, dh=64, hidden=3072):
  - Residual spine kept in fp32, FEATURE-major ([d, tokens]) so every matmul
    uses the natural [d_in, d_out] weight layout as lhsT (stationary).
  - All matmuls in bf16 (PE is 4x faster than fp32); accumulation fp32 PSUM.
  - LN gains/biases folded into qkv_w/fc1_w host-side -> device LN is pure
    (x - mu) * rstd.
  - LN1 stats token-major via bn_stats on the DMA-loaded x, moved to row
    layout via a tiny DRAM round-trip, broadcast via GPSIMD.
  - LN2 stats feature-major via ones-matmul (mean, mean-of-square).
  - Attention: scoresT[j,i] = K_h^T Q_h per head; exp via ACT (scale=1/8
    folded, no max-subtraction needed -- scores are bounded ~|2|);
    softmax denominator via an appended ones-column on V (token-major V
    computed directly from the qkv matmul); normalization by GPSIMD
    partition-broadcast of the reciprocal row.
  - PE transposes (identity matmul) only at entry (x -> x_fm) and exit.
  - SBUF reuse via same-tag tile chaining (Tile adds the anti-deps).
"""

import numpy as np
import ml_dtypes

import concourse.bass as bass
import concourse.tile as tile
from concourse import mybir
from concourse.masks import make_identity
from concourse.bass_utils import run_bass_kernel_spmd

F32 = mybir.dt.float32
BF16 = mybir.dt.bfloat16
BF16_NP = ml_dtypes.bfloat16
AF = mybir.ActivationFunctionType

N = 1024          # tokens per core
D = 768           # model dim
H = 12            # heads
DH = 64           # head dim
HID = 3072        # mlp hidden
P = 128
NT = N // P       # 8 token chunks
DC = D // P       # 6 feature chunks
HC = HID // P     # 24 hidden chunks
IC = N // 512     # 2 free-dim chunks of 512
EPS = 1e-5


def build_nc():
    nc = bass.Bass("TRN2")

    x_d = nc.dram_tensor("x", [N, D], F32, kind="ExternalInput")
    qkvw_d = nc.dram_tensor("qkv_w", [D, 3 * D], BF16, kind="ExternalInput")
    qkvb_d = nc.dram_tensor("qkv_b", [3 * D], F32, kind="ExternalInput")
    projw_d = nc.dram_tensor("proj_w", [D, D], BF16, kind="ExternalInput")
    projb_d = nc.dram_tensor("proj_b", [D], F32, kind="ExternalInput")
    fc1w_d = nc.dram_tensor("fc1_w", [D, HID], BF16, kind="ExternalInput")
    fc1b_d = nc.dram_tensor("fc1_b", [HID], F32, kind="ExternalInput")
    fc2w_d = nc.dram_tensor("fc2_w", [HID, D], BF16, kind="ExternalInput")
    fc2b_d = nc.dram_tensor("fc2_b", [D], F32, kind="ExternalInput")
    y_d = nc.dram_tensor("y", [N, D], F32, kind="ExternalOutput")

    with tile.TileContext(nc) as tc:
        _body(nc, tc, x_d, qkvw_d, qkvb_d, projw_d, projb_d,
              fc1w_d, fc1b_d, fc2w_d, fc2b_d, y_d)
    # this container's walrus accepts at most 1 sync wait per instruction
    # (2 on EventSemaphore); redistribute excess waits like Bacc.compile does
    import bass_rust as _br
    _br.move_matmul_waits_to_ldweights(nc.m)
    _br.generate_event_semaphores(nc)
    return nc


def _body(nc, tc, x_d, qkvw_d, qkvb_d, projw_d, projb_d,
          fc1w_d, fc1b_d, fc2w_d, fc2b_d, y_d):
    from contextlib import ExitStack
    with ExitStack() as ctx:
        consts = ctx.enter_context(tc.tile_pool(name="consts", bufs=1))
        stats = ctx.enter_context(tc.tile_pool(name="stats", bufs=1))
        bcp = ctx.enter_context(tc.tile_pool(name="bcp", bufs=2))
        rows = ctx.enter_context(tc.tile_pool(name="rows", bufs=3))
        dram = ctx.enter_context(tc.tile_pool(name="dram", bufs=1, space="DRAM"))
        ps_mm = ctx.enter_context(tc.tile_pool(name="ps_mm", bufs=2, space="PSUM"))
        mem = ctx.enter_context(tc.tile_pool(name="mem", bufs=1))
        mem2 = ctx.enter_context(tc.tile_pool(name="mem2", bufs=2))
        mem1 = ctx.enter_context(tc.tile_pool(name="mem1", bufs=1))

        # ---- constants ----
        ident = consts.tile([P, P], F32, tag="ident")
        make_identity(nc, ident)
        x_tok = mem.tile([P, NT, D], F32, tag="xo")            # -> out_fm later
        x_view = x_d[:, :].rearrange("(t p) d -> p t d", p=P)
        for t in range(NT):
            nc.sync.dma_start(out=x_tok[:, t, :], in_=x_view[:, t, :])
        ones_bf = consts.tile([P, 1], BF16, tag="ones_bf")
        nc.vector.memset(ones_bf, 1.0)
        eps_sb = consts.tile([P, 1], F32, tag="eps_sb")
        nc.vector.memset(eps_sb, EPS)
        ones_row = consts.tile([1, P], F32, tag="ones_row")
        nc.vector.memset(ones_row, 1.0)
        ones_f32h = consts.tile([P, 1], F32, tag="ones_f32h")
        nc.vector.memset(ones_f32h, 1.0)

        def dma_bcast(dst, dram_ap):
            """dst [K, M] sbuf <- partition-broadcast of dram_ap [M] (DRAM).
            Rides SWDGE so it never queues behind bulk weight loads."""
            kk = dst.shape[0]
            nc.sync.dma_start(out=dst, in_=bass.AP(
                tensor=dram_ap.tensor, offset=dram_ap.offset,
                ap=[[0, kk], *dram_ap.ap]))

        # biases: feature-major per-partition layout [P, chunks]
        def load_bias_pm(dram_ap, nchunks, tag):
            t = consts.tile([P, nchunks], F32, tag=tag)
            nc.sync.dma_start(out=t, in_=dram_ap.rearrange("(mo p) -> p mo", p=P))
            return t

        qkb_sb = load_bias_pm(qkvb_d[0:2 * D], 2 * DC, "qkb")      # q,k bias
        projb_sb = load_bias_pm(projb_d[:], DC, "projb")
        fc1b_sb = load_bias_pm(fc1b_d[:], HC, "fc1b")
        fc2b_sb = load_bias_pm(fc2b_d[:], DC, "fc2b")
        # v bias broadcast across partitions [P, D]
        vb_bc = mem1.tile([P, D], F32, tag="m1")
        dma_bcast(vb_bc, qkvb_d[2 * D:3 * D])

        # ---- phase 0: remaining loads ----
        qkvw_sb = mem.tile([P, DC, 3 * D], BF16, tag="w1")     # -> fc1w later
        qkvw_v = qkvw_d[:, :].rearrange("(ko p) m -> p ko m", p=P)
        for k in range(DC):
            nc.sync.dma_start(out=qkvw_sb[:, k, :], in_=qkvw_v[:, k, :])
        projw_sb = mem.tile([P, DC, D], BF16, tag="pw")        # -> x2_sq later

        # ---- phase 1: LN1 stats (token-major bn_stats) ----
        mu1 = stats.tile([P, NT], F32, tag="mu1")
        rstd1 = stats.tile([P, NT], F32, tag="rstd1")
        for t in range(NT):
            xg = x_tok[:, t, :].rearrange("p (s c) -> p s c", c=256)
            st = stats.tile([P, 3, 6], F32, tag="bnst")
            for s in range(3):
                nc.vector.bn_stats(out=st[:, s, :], in_=xg[:, s, :])
            mv = stats.tile([P, 2], F32, tag="bnmv")
            nc.vector.bn_aggr(out=mv, in_=st)
            nc.vector.tensor_copy(out=mu1[:, t:t + 1], in_=mv[:, 0:1])
            nc.scalar.activation(out=rstd1[:, t:t + 1], in_=mv[:, 1:2],
                                 func=AF.Sqrt, bias=eps_sb, scale=1.0)
        nc.vector.reciprocal(out=rstd1[:, 0:4], in_=rstd1[:, 0:4])
        nc.vector.reciprocal(out=rstd1[:, 4:8], in_=rstd1[:, 4:8])

        # stats -> row layout via DRAM round-trip, then partition-broadcast,
        # in token halves so downstream work starts on half 0 early
        sdram = dram.tile([4, N], F32, tag="sdram")
        mu1_bc = bcp.tile([P, N], F32, tag="bc")
        rstd1_bc = bcp.tile([P, N], F32, tag="bc")
        for hf in range(2):
            sl = slice(hf * 512, (hf + 1) * 512)
            tt = slice(hf * 4, (hf + 1) * 4)
            nc.sync.dma_start(
                out=sdram[0, sl].rearrange("(t p) -> p t", p=P), in_=mu1[:, tt])
            nc.sync.dma_start(
                out=sdram[1, sl].rearrange("(t p) -> p t", p=P), in_=rstd1[:, tt])
            dma_bcast(mu1_bc[:, sl], sdram[0, sl])
            dma_bcast(rstd1_bc[:, sl], sdram[1, sl])

        # ---- phase 2: x -> feature-major via PE transpose ----
        x_fm = mem.tile([P, DC, N], F32, tag="xg")             # -> gelu later
        with tc.tile_pool(name="ps_tr1", bufs=3, space="PSUM") as ps_tr1:
            for dc in range(DC):
                for tg in range(NT // 4):
                    pt = ps_tr1.tile([P, 4, P], F32, tag="tr")
                    for q in range(4):
                        t = tg * 4 + q
                        nc.tensor.transpose(pt[:, q, :],
                                            x_tok[:, t, dc * P:(dc + 1) * P],
                                            ident)
                    eng = nc.scalar.copy if tg % 2 else nc.vector.tensor_copy
                    eng(out=x_fm[:, dc, tg * 4 * P:(tg + 1) * 4 * P],
                        in_=pt.rearrange("p a b -> p (a b)"))

        # ---- phase 3: LN1 apply -> h1 (bf16, feature-major, in-place 2-op) ----
        h1 = mem.tile([P, DC, N], BF16, tag="ha")              # -> attn, h2 later
        for hf in range(2):
            sl = slice(hf * 512, (hf + 1) * 512)
            for dc in range(DC):
                eng = nc.gpsimd if dc >= 4 else nc.vector
                eng.tensor_sub(h1[:, dc, sl], x_fm[:, dc, sl], mu1_bc[:, sl])
                eng.tensor_mul(h1[:, dc, sl], h1[:, dc, sl], rstd1_bc[:, sl])

        # ---- phase 4: q, k matmuls, emitted pairwise inside the head loop ----
        qk_fm = mem.tile([P, 2 * DC, N], BF16, tag="qx")       # -> x2 later

        def emit_qk_pair(pair):
            for mo in (pair, DC + pair):
                for ic in range(IC):
                    ps = ps_mm.tile([P, 512], F32, tag="mm", name="ps_qk")
                    for k in range(DC):
                        nc.tensor.matmul(ps, qkvw_sb[:, k, mo * P:(mo + 1) * P],
                                         h1[:, k, ic * 512:(ic + 1) * 512],
                                         start=(k == 0), stop=(k == DC - 1))
                    nc.vector.tensor_scalar_add(
                        out=qk_fm[:, mo, ic * 512:(ic + 1) * 512],
                        in0=ps, scalar1=qkb_sb[:, mo:mo + 1])

        emit_qk_pair(0)

        # ---- phase 5: v matmul (token-major out, augmented with ones col) ----
        v_aug = mem.tile([P, NT, H, DH + 1], BF16, tag="vf")   # -> fc2w later
        nc.vector.memset(v_aug[:, :, :, DH:DH + 1], 1.0)

        def emit_v_half(fc):
            fw = 512 if fc == 0 else 256
            for t in range(NT):
                ps = ps_mm.tile([P, 512], F32, tag="mm", name="ps_v")
                for k in range(DC):
                    nc.tensor.matmul(ps[:, :fw],
                                     h1[:, k, t * P:(t + 1) * P],
                                     qkvw_sb[:, k, 2 * D + fc * 512:2 * D + fc * 512 + fw],
                                     start=(k == 0), stop=(k == DC - 1))
                nh = fw // DH
                nc.vector.tensor_add(
                    out=v_aug[:, t, fc * 8:fc * 8 + nh, 0:DH],
                    in0=ps[:, :fw].rearrange("p (h e) -> p h e", e=DH),
                    in1=vb_bc[:, fc * 512:fc * 512 + fw].rearrange(
                        "p (h e) -> p h e", e=DH))

        nc.sync.dma_start(out=projw_sb,
                          in_=projw_d[:, :].rearrange("(ko p) m -> p ko m", p=P))
        # late weights (reuse released zones via tag chaining); the DMAs are
        # emitted inside the head loop so they ride the DMA-idle window
        fc1w_sb = mem.tile([P, DC, HID], BF16, tag="w1")
        fc1w_v = fc1w_d[:, :].rearrange("(ko p) m -> p ko m", p=P)

        def emit_fc1w_chunk(k):
            for hlf in range(2):
                nc.sync.dma_start(
                    out=fc1w_sb[:, k, hlf * HID // 2:(hlf + 1) * HID // 2],
                    in_=fc1w_v[:, k, hlf * HID // 2:(hlf + 1) * HID // 2])

        # ---- phase 6: attention, head x column-half ----
        attn_fm = mem.tile([P, DC, N], BF16, tag="ha")
        rdram = dram.tile([H, IC, 512], F32, tag="rdram")
        with tc.tile_pool(name="ps_sc", bufs=2, space="PSUM") as ps_sc, \
             tc.tile_pool(name="ps_att", bufs=2, space="PSUM") as ps_att:
            def emit_scores(h, ic):
                po = 64 * (h % 2)
                q_h = qk_fm[po:po + 64, h // 2, :]
                k_h = qk_fm[po:po + 64, DC + h // 2, :]
                expT = mem2.tile([P, NT, 512], BF16, tag="ex", name="expT")
                for jp in range(NT // 2):
                    ps = ps_sc.tile([P, 2, 512], F32, tag="sc", name="ps_sc")
                    for half in range(2):
                        jc = 2 * jp + half
                        nc.tensor.matmul(ps[:, half, :],
                                         k_h[:, jc * P:(jc + 1) * P],
                                         q_h[:, ic * 512:(ic + 1) * 512],
                                         start=True, stop=True)
                    nc.scalar.activation(
                        out=expT[:, 2 * jp:2 * jp + 2, :], in_=ps,
                        func=AF.Exp, scale=0.125)
                return expT

            def emit_attnv(h, ic, expT):
                po = 64 * (h % 2)
                pso = ps_att.tile([DH + 1, 512], F32, tag="att", name="pso")
                for jc in range(NT):
                    nc.tensor.matmul(pso, v_aug[:, jc, h, :],
                                     expT[:, jc, :],
                                     start=(jc == 0), stop=(jc == NT - 1))
                rec = mem1.tile([1, 512], F32, tag="rec1", name="rec")
                nc.vector.reciprocal(out=rec, in_=pso[DH:DH + 1, :])
                rb = mem2.tile([DH, 512], F32, tag="rb", name="rb")
                if h >= H - 2:
                    psb = ps_mm.tile([P, 512], F32, tag="mm", name="psb")
                    nc.tensor.matmul(psb[:DH, :], ones_row[:, :DH], rec,
                                     start=True, stop=True)
                    nc.vector.tensor_copy(out=rb, in_=psb[:DH, :])
                else:
                    nc.sync.dma_start(out=rdram[h, ic, :], in_=rec)
                    dma_bcast(rb, rdram[h, ic, :])
                nc.vector.tensor_mul(
                    out=attn_fm[po:po + 64, h // 2, ic * 512:(ic + 1) * 512],
                    in0=pso[0:DH, :], in1=rb)

            for h in range(H):
                if 5 <= h < 11:
                    emit_fc1w_chunk(h - 5)
                if h == 0:
                    emit_qk_pair(1)
                    e0 = emit_scores(0, 0)
                    emit_v_half(0)
                    emit_attnv(0, 0, e0)
                    e1 = emit_scores(0, 1)
                    emit_v_half(1)
                    emit_attnv(0, 1, e1)
                else:
                    for ic in range(IC):
                        e = emit_scores(h, ic)
                        if ic == 0 and h % 2 == 0 and h // 2 + 1 < DC:
                            emit_qk_pair(h // 2 + 1)
                        emit_attnv(h, ic, e)

        fc2w_a = mem.tile([P, HC // 2, D], BF16, tag="vf")
        fc2w_av = fc2w_d[0:HID // 2, :].rearrange("(ko p) m -> p ko m", p=P)
        for k4 in range(4):
            nc.sync.dma_start(out=fc2w_a[:, k4 * 3:(k4 + 1) * 3, :],
                              in_=fc2w_av[:, k4 * 3:(k4 + 1) * 3, :])

        # ---- phase 7+8: proj + residual -> x2, with LN2 stats interleaved ----
        x2_fm = mem.tile([P, DC, N], F32, tag="qx")
        x2_sq = mem.tile([P, DC, N], BF16, tag="pw")
        mu2_row = rows.tile([1, N], F32, tag="row")
        s2_row = rows.tile([1, N], F32, tag="row")
        ps_row_cm = tc.tile_pool(name="ps_row", bufs=4, space="PSUM")
        ps_row = ps_row_cm.__enter__()
        ps_mu = [ps_row.tile([1, 512], F32, tag="row", name=f"ps_mu{i}")
                 for i in range(IC)]
        ps_s2 = [ps_row.tile([1, 512], F32, tag="row", name=f"ps_s2{i}")
                 for i in range(IC)]
        for mo in range(DC):
            for ic in range(IC):
                ps = ps_mm.tile([P, 512], F32, tag="mm")
                for k in range(DC):
                    nc.tensor.matmul(ps, projw_sb[:, k, mo * P:(mo + 1) * P],
                                     attn_fm[:, k, ic * 512:(ic + 1) * 512],
                                     start=(k == 0), stop=(k == DC - 1))
                nc.vector.scalar_tensor_tensor(
                    out=x2_fm[:, mo, ic * 512:(ic + 1) * 512],
                    in0=ps, scalar=projb_sb[:, mo:mo + 1],
                    in1=x_fm[:, mo, ic * 512:(ic + 1) * 512],
                    op0=mybir.AluOpType.add, op1=mybir.AluOpType.add)
            # LN2 stats for this chunk: cast x2->bf16 into the x2_sq
            # buffer, take the mean from it, then square it in place
            nc.gpsimd.tensor_copy(out=x2_sq[:, mo, 0:512],
                                  in_=x2_fm[:, mo, 0:512])
            nc.vector.tensor_copy(out=x2_sq[:, mo, 512:1024],
                                  in_=x2_fm[:, mo, 512:1024])
            for ic in range(IC):
                nc.tensor.matmul(ps_mu[ic], ones_bf,
                                 x2_sq[:, mo, ic * 512:(ic + 1) * 512],
                                 start=(mo == 0), stop=(mo == DC - 1))
            nc.gpsimd.tensor_mul(x2_sq[:, mo, 0:512], x2_sq[:, mo, 0:512],
                                 x2_sq[:, mo, 0:512])
            nc.vector.tensor_mul(x2_sq[:, mo, 512:1024],
                                 x2_sq[:, mo, 512:1024],
                                 x2_sq[:, mo, 512:1024])
            for ic in range(IC):
                nc.tensor.matmul(ps_s2[ic], ones_bf,
                                 x2_sq[:, mo, ic * 512:(ic + 1) * 512],
                                 start=(mo == 0), stop=(mo == DC - 1))
        for ic in range(IC):
            nc.scalar.mul(out=mu2_row[0:1, ic * 512:(ic + 1) * 512],
                          in_=ps_mu[ic], mul=1.0 / D)
            nc.scalar.mul(out=s2_row[0:1, ic * 512:(ic + 1) * 512],
                          in_=ps_s2[ic], mul=1.0 / D)
        ps_row_cm.__exit__(None, None, None)
        # var = E[x^2] - mu^2 ; rstd = 1/sqrt(var+eps)
        var_row = rows.tile([1, N], F32, tag="row")
        nc.vector.tensor_mul(var_row, mu2_row, mu2_row)
        nc.vector.tensor_sub(var_row, s2_row, var_row)
        rstd2_row = rows.tile([1, N], F32, tag="row")
        nc.scalar.activation(out=rstd2_row, in_=var_row,
                             func=AF.Sqrt, bias=eps_sb[0:1, :], scale=1.0)
        nc.vector.reciprocal(out=rstd2_row, in_=rstd2_row)
        nc.sync.dma_start(out=sdram[2:3, :], in_=mu2_row)
        nc.sync.dma_start(out=sdram[3:4, :], in_=rstd2_row)
        mu2_bc = bcp.tile([P, N], F32, tag="bc")
        rstd2_bc = bcp.tile([P, N], F32, tag="bc")
        dma_bcast(mu2_bc, sdram[2, :])
        dma_bcast(rstd2_bc, sdram[3, :])

        fc2w_b = mem.tile([P, HC // 2, D], BF16, tag="pw")
        fc2w_bv = fc2w_d[HID // 2:, :].rearrange("(ko p) m -> p ko m", p=P)
        for k4 in range(4):
            nc.sync.dma_start(out=fc2w_b[:, k4 * 3:(k4 + 1) * 3, :],
                              in_=fc2w_bv[:, k4 * 3:(k4 + 1) * 3, :])

        # LN2 apply -> h2
        h2 = mem.tile([P, DC, N], BF16, tag="ha")
        for hf in range(2):
            sl = slice(hf * 512, (hf + 1) * 512)
            for dc in range(DC):
                eng = nc.gpsimd if dc >= 4 else nc.vector
                eng.tensor_sub(h2[:, dc, sl], x2_fm[:, dc, sl], mu2_bc[:, sl])
                eng.tensor_mul(h2[:, dc, sl], h2[:, dc, sl], rstd2_bc[:, sl])

        # ---- phases 9-11: MLP in column halves, with output transpose ----
        out_fm = mem.tile([P, DC, N], F32, tag="xo")
        ps_tr2_cm = tc.tile_pool(name="ps_tr2", bufs=3, space="PSUM")
        ps_tr2 = ps_tr2_cm.__enter__()
        for ic in range(IC):
            gelu = mem.tile([P, HC, 512], BF16, tag="xg")
            for mo in range(HC):
                ps = ps_mm.tile([P, 512], F32, tag="mm")
                for k in range(DC):
                    nc.tensor.matmul(ps, fc1w_sb[:, k, mo * P:(mo + 1) * P],
                                     h2[:, k, ic * 512:(ic + 1) * 512],
                                     start=(k == 0), stop=(k == DC - 1))
                nc.scalar.activation(out=gelu[:, mo, :], in_=ps,
                                     func=AF.Gelu,
                                     bias=fc1b_sb[:, mo:mo + 1], scale=1.0)
            for mo in range(DC):
                ps = ps_mm.tile([P, 512], F32, tag="mm")
                for k in range(HC):
                    wsl = (fc2w_a[:, k, mo * P:(mo + 1) * P] if k < HC // 2
                           else fc2w_b[:, k - HC // 2, mo * P:(mo + 1) * P])
                    nc.tensor.matmul(ps, wsl, gelu[:, k, :],
                                     start=(k == 0), stop=(k == HC - 1))
                nc.vector.scalar_tensor_tensor(
                    out=out_fm[:, mo, ic * 512:(ic + 1) * 512],
                    in0=ps, scalar=fc2b_sb[:, mo:mo + 1],
                    in1=x2_fm[:, mo, ic * 512:(ic + 1) * 512],
                    op0=mybir.AluOpType.add, op1=mybir.AluOpType.add)
            # transpose + store this column half's token chunks
            for t in range(ic * NT // IC, (ic + 1) * NT // IC):
                y_stage = mem1.tile([P, D], F32, tag=('m1' if t % 2 else 'ys2'),
                                    name='y_stage')
                for dg in range(2):
                    pt = ps_tr2.tile([P, 3, P], F32, tag="tr2")
                    for q in range(3):
                        dc = dg * 3 + q
                        nc.tensor.transpose(pt[:, q, :],
                                            out_fm[:, dc, t * P:(t + 1) * P],
                                            ident)
                    eng = nc.scalar.copy if (t + dg) % 2 else nc.vector.tensor_copy
                    eng(out=y_stage[:, dg * 3 * P:(dg + 1) * 3 * P],
                        in_=pt.rearrange("p a b -> p (a b)"))
                nc.sync.dma_start(out=y_d[t * P:(t + 1) * P, :], in_=y_stage)
        ps_tr2_cm.__exit__(None, None, None)


_NC_CACHE = {}


def _get_nc():
    if "nc" not in _NC_CACHE:
        _NC_CACHE["nc"] = build_nc()
    return _NC_CACHE["nc"]


def _prep_inputs(x, ln1_g, ln1_b, qkv_w, qkv_b, proj_w, proj_b,
                 ln2_g, ln2_b, fc1_w, fc1_b, fc2_w, fc2_b):
    f = lambda a: np.asarray(a, np.float32)
    x = f(x)
    qkv_w, qkv_b = f(qkv_w), f(qkv_b)
    proj_w, proj_b = f(proj_w), f(proj_b)
    fc1_w, fc1_b = f(fc1_w), f(fc1_b)
    fc2_w, fc2_b = f(fc2_w), f(fc2_b)
    ln1_g, ln1_b, ln2_g, ln2_b = f(ln1_g), f(ln1_b), f(ln2_g), f(ln2_b)

    # fold LN affine into the following matmul
    qkv_w_eff = (ln1_g[:, None] * qkv_w).astype(BF16_NP)
    qkv_b_eff = (qkv_b + ln1_b @ qkv_w).astype(np.float32)
    fc1_w_eff = (ln2_g[:, None] * fc1_w).astype(BF16_NP)
    fc1_b_eff = (fc1_b + ln2_b @ fc1_w).astype(np.float32)

    shared = {
        "qkv_w": qkv_w_eff, "qkv_b": qkv_b_eff,
        "proj_w": proj_w.astype(BF16_NP), "proj_b": proj_b,
        "fc1_w": fc1_w_eff, "fc1_b": fc1_b_eff,
        "fc2_w": fc2_w.astype(BF16_NP), "fc2_b": fc2_b,
    }
    n_cores = x.shape[0]
    return [{"x": np.ascontiguousarray(x[c]), **shared} for c in range(n_cores)]


def kernel(**inputs):
    in_maps = _prep_inputs(**inputs)
    nc = _get_nc()
    res = run_bass_kernel_spmd(nc, in_maps, core_ids=list(range(len(in_maps))))
    return np.stack([r["y"] for r in res.results], axis=0)


if __name__ == "__main__":
    import reference
    inputs = {k: np.asarray(v) for k, v in reference.setup_inputs().items()}
    out = kernel(**inputs)
    print("kernel out", out.shape, out.dtype)



# revision 41
# speedup vs baseline: 1.4095x; 1.4095x over previous
"""Trainium2 Bass kernel for a dense transformer block (PreNorm attn + MLP).

Full inputs: x [8, 1024, 768] f32 + LN/attn/MLP weights.
Sharding: pure data-parallel — batch 8 across 8 NeuronCores, no collectives.

Per-core design (tokens n=1024, d=768, heads=12, dh=64, hidden=3072):
  - Residual spine fp32 (x2) / bf16 (x), FEATURE-major; weights [d_in, d_out]
    serve as lhsT directly.
  - All deep matmuls fp8e4 DoubleRow: one instruction contracts TWO 128-deep
    k-tiles at 0.5 cycles/moving-row.
  - Scores: q/k "folded" — head h on 32 partitions, dh split into the 2
    DoubleRow k-tiles (host-side qkv_w column permutation makes this free).
  - attnV: lhsT = v_aug [j, 2, 96]: 64 v dims + ones col (softmax
    denominator) + 31 zero pad (DoubleRow needs M % 32 == 0). The v bias
    commutes through softmax (weights sum to 1) and is folded into proj_b
    host-side.
  - exp on ACT is THE bottleneck (~100us); everything else is scheduled to
    hide under it: token-half (ic) outer / heads inner; a work queue drains
    one MLP/qkv chunk per scores-pair so the in-order PE queue never blocks
    on a pending PSUM drain.
  - LN stats feature-major: bf16 ones-matmuls (LN1) / fp8 DoubleRow
    ones-matmuls (LN2), Rsqrt on ACT, PE ones-broadcast into PSUM (LN1) or
    gpsimd partition_broadcast (LN2).
  - Tail (ic1 MLP) runs at token-quarter granularity to pipeline stages.
"""

import collections

import numpy as np
import ml_dtypes

import concourse.bass as bass
import concourse.tile as tile
from concourse import mybir
from concourse.masks import make_identity
from concourse.bass_utils import run_bass_kernel_spmd

F32 = mybir.dt.float32
BF16 = mybir.dt.bfloat16
FP8 = mybir.dt.float8e4
BF16_NP = ml_dtypes.bfloat16
FP8_NP = ml_dtypes.float8_e4m3
AF = mybir.ActivationFunctionType
DR = mybir.MatmulPerfMode.DoubleRow
ADD = mybir.AluOpType.add

N = 1024          # tokens per core
D = 768           # model dim
H = 12            # heads
DH = 64           # head dim
HID = 3072        # mlp hidden
P = 128
NT = N // P       # 8 token chunks
DC = D // P       # 6 feature chunks
HC = HID // P     # 24 hidden chunks
KP = DC // 2      # 3 contraction pairs for d=768
IC = 2            # token halves of 512
VA = 96           # attnV stationary cols: 64 v + ones + 31 pad
EPS = 1e-5


def build_nc(qkb_zero=False):
    nc = bass.Bass("TRN2")

    x_d = nc.dram_tensor("x", [N, D], F32, kind="ExternalInput")
    wqkv_d = nc.dram_tensor("wqkv", [D, 3 * D], FP8, kind="ExternalInput")
    qkb_d = nc.dram_tensor("qkb", [2 * D], F32, kind="ExternalInput")
    projw_d = nc.dram_tensor("proj_w", [D, D], FP8, kind="ExternalInput")
    wsum_d = nc.dram_tensor("wsum8", [D, 32], FP8, kind="ExternalInput")
    pbsum_d = nc.dram_tensor("pbsum", [1], F32, kind="ExternalInput")
    projb_d = nc.dram_tensor("proj_b", [D], F32, kind="ExternalInput")
    fc1w_d = nc.dram_tensor("fc1_w", [D, HID], FP8, kind="ExternalInput")
    fc1b_d = nc.dram_tensor("fc1_b", [HID], F32, kind="ExternalInput")
    fc2w_d = nc.dram_tensor("fc2_w", [HID, D], FP8, kind="ExternalInput")
    fc2b_d = nc.dram_tensor("fc2_b", [D], F32, kind="ExternalInput")
    y_d = nc.dram_tensor("y", [N, D], F32, kind="ExternalOutput")
    nc._dbg = {
        "attn": nc.dram_tensor("dbg_attn", [P, DC, N], FP8,
                               kind="ExternalOutput"),
        "x2": nc.dram_tensor("dbg_x2", [P, DC, N], F32,
                             kind="ExternalOutput"),
        "h1": nc.dram_tensor("dbg_h1", [P, DC, N], FP8,
                             kind="ExternalOutput"),
        "h2a": nc.dram_tensor("dbg_h2a", [P, DC, 512], FP8,
                              kind="ExternalOutput"),
        "mux": nc.dram_tensor("dbg_mux", [1, N], F32,
                              kind="ExternalOutput"),
    }

    with tile.TileContext(nc) as tc:
        _body(nc, tc, x_d, wqkv_d, qkb_d, projw_d, projb_d,
              fc1w_d, fc1b_d, fc2w_d, fc2b_d, y_d, qkb_zero,
              wsum_d, pbsum_d)
    # this container's walrus accepts at most 1 sync wait per instruction
    # (2 on EventSemaphore); redistribute excess waits like Bacc.compile does
    import bass_rust as _br
    _br.move_matmul_waits_to_ldweights(nc.m)
    _br.generate_event_semaphores(nc)
    return nc


def _body(nc, tc, x_d, wqkv_d, qkb_d, projw_d, projb_d,
          fc1w_d, fc1b_d, fc2w_d, fc2b_d, y_d, qkb_zero,
          wsum_d, pbsum_d):
    from contextlib import ExitStack
    with ExitStack() as ctx:
        consts = ctx.enter_context(tc.tile_pool(name="consts", bufs=1))
        rows = ctx.enter_context(tc.tile_pool(name="rows", bufs=3))
        recp = ctx.enter_context(tc.tile_pool(name="recp", bufs=2))
        rbp = ctx.enter_context(tc.tile_pool(name="rbp", bufs=2))
        ltp = ctx.enter_context(tc.tile_pool(name="ltp", bufs=2))
        bcp = ctx.enter_context(tc.tile_pool(name="bcp", bufs=2))
        dram = ctx.enter_context(tc.tile_pool(name="dram", bufs=1, space="DRAM"))
        ps_mm = ctx.enter_context(tc.tile_pool(name="ps_mm", bufs=2, space="PSUM"))
        mem = ctx.enter_context(tc.tile_pool(name="mem", bufs=1))
        mem2 = ctx.enter_context(tc.tile_pool(name="mem2", bufs=2))

        ALP = nc.allow_low_precision

        # ---- constants & early DMAs (x first, then q/k weights) ----
        ident = consts.tile([P, P], F32, tag="ident")
        make_identity(nc, ident)
        x_tok = mem.tile([P, NT, D], F32, tag="xo")            # -> out_fm later
        x_view = x_d[:, :].rearrange("(t p) d -> p t d", p=P)
        for t in range(4):
            for c in range(2):
                nc.sync.dma_start(out=x_tok[:, t, 384 * c:384 * (c + 1)],
                                  in_=x_view[:, t, 384 * c:384 * (c + 1)])

        wqkv_sb = mem.tile([P, DC, 3 * D], FP8, tag="w1")      # -> fc1w later
        wqkv_v = wqkv_d[:, :].rearrange("(ko p) m -> p ko m", p=P)
        for k in range(DC):
            for c in range(2):
                nc.sync.dma_start(out=wqkv_sb[:, k, D * c:D * (c + 1)],
                                  in_=wqkv_v[:, k, D * c:D * (c + 1)])

        ones8 = consts.tile([P, 2, 32], FP8, tag="ones8")
        nc.vector.memset(ones8, 1.0)
        onesb_c = consts.tile([P, 1], BF16, tag="onesb_c")     # stats lhsT
        nc.vector.memset(onesb_c, 1.0)
        onesb_r = consts.tile([1, P], BF16, tag="onesb_r")     # bcast lhsT
        nc.vector.memset(onesb_r, 1.0)
        eps_sb = consts.tile([P, 1], F32, tag="eps_sb")
        nc.vector.memset(eps_sb, EPS)

        def dma_bcast(dst, dram_ap):
            kk = dst.shape[0]
            nc.sync.dma_start(out=dst, in_=bass.AP(
                tensor=dram_ap.tensor, offset=dram_ap.offset,
                ap=[[0, kk], *dram_ap.ap]))

        def load_bias_pm(dram_ap, nchunks, tag):
            t = consts.tile([P, nchunks], F32, tag=tag)
            nc.sync.dma_start(out=t, in_=dram_ap.rearrange("(mo p) -> p mo", p=P))
            return t

        qkb_sb = load_bias_pm(qkb_d[:], 2 * DC, "qkb")
        projb_sb = load_bias_pm(projb_d[:], DC, "projb")
        fc1b_sb = load_bias_pm(fc1b_d[:], HC, "fc1b")
        fc2b_sb = load_bias_pm(fc2b_d[:], DC, "fc2b")

        # deferred weight DMAs (after x-half0 / qk in the DMA queues)
        for k in range(DC):
            nc.sync.dma_start(out=wqkv_sb[:, k, 2 * D:],
                              in_=wqkv_v[:, k, 2 * D:])
        for t in range(4, NT):
            for c in range(2):
                nc.sync.dma_start(out=x_tok[:, t, 384 * c:384 * (c + 1)],
                                  in_=x_view[:, t, 384 * c:384 * (c + 1)])
        wsum_sb = consts.tile([P, DC, 32], FP8, tag="wsum")
        nc.sync.dma_start(out=wsum_sb,
                          in_=wsum_d[:, :].rearrange("(ko p) m -> p ko m", p=P))
        pbs_sb = consts.tile([1, 1], F32, tag="pbs")
        nc.sync.dma_start(out=pbs_sb, in_=pbsum_d[:])
        projw_sb = mem.tile([P, DC, D], FP8, tag="pw")
        projw_v = projw_d[:, :].rearrange("(ko p) m -> p ko m", p=P)
        for k in range(DC):
            nc.sync.dma_start(out=projw_sb[:, k, :], in_=projw_v[:, k, :])

        # v_aug: ones col + zero pad (finite garbage would still poison psum)
        v_aug = mem.tile([P, NT, H, VA], FP8, tag="vf")
        nc.vector.memset(v_aug[:, :, :, DH + 1:], 0.0)
        nc.vector.memset(v_aug[:, :, :, DH:DH + 1], 1.0)

        # ---- lead-in helpers: transposes + feature-major LN1 ----
        x_fm = mem.tile([P, DC, N], BF16, tag="xf")
        xsq = mem.tile([P, DC, 512], BF16, tag="xq")
        muxr = mem.tile([1, N], BF16, tag="mux")
        h1 = mem.tile([P, DC, N], FP8, tag="ha")               # -> h2 later
        bc_sb = [None, None]

        def lead_tr(hf, dc, pool, ptag):
            sl = slice(hf * 512, (hf + 1) * 512)
            pt = pool.tile([P, 4, P], F32, tag=ptag)
            for q in range(4):
                t = hf * 4 + q
                nc.tensor.transpose(pt[:, q, :],
                                    x_tok[:, t, dc * P:(dc + 1) * P], ident)
            eng = nc.scalar.copy if dc % 2 else nc.vector.tensor_copy
            with ALP(reason="bf16 x_fm"):
                eng(out=x_fm[:, dc, sl], in_=pt.rearrange("p a b -> p (a b)"))
            with ALP(reason="bf16 xsq"):
                eng2 = nc.gpsimd if dc % 2 else nc.vector
                eng2.tensor_mul(xsq[:, dc, :], x_fm[:, dc, sl],
                                x_fm[:, dc, sl])

        def lead_stats(hf, pool, ptag):
            sl = slice(hf * 512, (hf + 1) * 512)
            psm = pool.tile([1, 512], F32, tag=ptag, name="ps_mu1")
            for k in range(DC):
                nc.tensor.matmul(psm, onesb_c, x_fm[:, k, sl],
                                 start=(k == 0), stop=(k == DC - 1))
            murow = muxr[0:1, sl]
            with ALP(reason="bf16 mux"):
                nc.vector.tensor_scalar_mul(out=murow, in0=psm,
                                            scalar1=1.0 / D)
            pss = pool.tile([1, 512], F32, tag=ptag, name="ps_s21")
            for k in range(DC):
                nc.tensor.matmul(pss, onesb_c, xsq[:, k, :],
                                 start=(k == 0), stop=(k == DC - 1))
            var = rows.tile([1, 512], F32, tag="row", name="var1")
            nc.vector.tensor_mul(var, murow, murow)
            v2 = rows.tile([1, 512], F32, tag="row", name="v2")
            nc.vector.tensor_scalar_mul(out=v2, in0=pss, scalar1=1.0 / D)
            nc.vector.tensor_sub(var, v2, var)
            rstd = rows.tile([1, 512], F32, tag="row", name="rstd1")
            nc.scalar.activation(out=rstd, in_=var, func=AF.Sqrt,
                                 bias=eps_sb[0:1, :], scale=1.0)
            nc.vector.reciprocal(out=rstd, in_=rstd)
            mu_bf = rows.tile([1, 512], BF16, tag="rowb", name="mu1bf")
            rs_bf = rows.tile([1, 512], BF16, tag="rowb", name="rs1bf")
            with ALP(reason="bf16 rows"):
                nc.vector.tensor_copy(out=mu_bf, in_=murow)
                nc.vector.tensor_copy(out=rs_bf, in_=rstd)
            mu_ps = pool.tile([P, 512], F32, tag=ptag, name="mu1ps")
            nc.tensor.matmul(mu_ps, onesb_r, mu_bf, start=True, stop=True)
            mu_bc = bcp.tile([P, 512], BF16, tag="bc", name="mu1bc")
            with ALP(reason="bf16 bc"):
                nc.vector.tensor_copy(out=mu_bc, in_=mu_ps)
            rs_ps = pool.tile([P, 512], F32, tag=ptag, name="rs1ps")
            nc.tensor.matmul(rs_ps, onesb_r, rs_bf, start=True, stop=True)
            rs_bc = bcp.tile([P, 512], BF16, tag="bc", name="rs1bc")
            with ALP(reason="bf16 bc"):
                nc.vector.tensor_copy(out=rs_bc, in_=rs_ps)
            bc_sb[hf] = (mu_bc, rs_bc)

        def lead_ln1(hf, dcs):
            sl = slice(hf * 512, (hf + 1) * 512)
            mu_bc, rs_bc = bc_sb[hf]
            for dc in dcs:
                eng = nc.gpsimd if dc in (2, 5) else nc.vector
                lt = ltp.tile([P, 512], BF16, tag="lt", name="ln1_tmp")
                with ALP(reason="ln1"):
                    eng.tensor_sub(lt, x_fm[:, dc, sl], mu_bc)
                    eng.tensor_mul(h1[:, dc, sl], lt, rs_bc)

        # ---- lead-in inline: attention needs ALL keys, so both halves ----
        with tc.tile_pool(name="ps_lead", bufs=3, space="PSUM") as ps_lead:
            for hf in range(2):
                for dc in range(DC):
                    lead_tr(hf, dc, ps_lead, "tr")
                lead_stats(hf, ps_lead, "tr")
                lead_ln1(hf, range(DC))

        # ---- DoubleRow helpers ----
        def dr_group(ps_ap, lhs_fn, rhs_fn, nkp):
            for kp in range(nkp):
                nc.tensor.matmul(ps_ap, lhs_fn(kp), rhs_fn(kp),
                                 start=(kp == 0), stop=(kp == nkp - 1),
                                 perf_mode=DR)

        q_fold = mem.tile([P, 3, 2, N], FP8, tag="qf")
        k_fold = mem.tile([P, 3, 2, N], FP8, tag="kf")

        def emit_qk_chunk(j, ic):
            """j in 0..11: q chunks 0-5 as (g, half), k chunks 6-11."""
            g, half = divmod(j % 6, 2)
            dst = k_fold if j >= 6 else q_fold
            sl = slice(ic * 512, (ic + 1) * 512)
            ps = ps_mm.tile([P, 512], F32, tag="mm", name="ps_qk")
            dr_group(ps,
                     lambda kp: wqkv_sb[:, 2 * kp:2 * kp + 2, j * P:(j + 1) * P],
                     lambda kp: h1[:, 2 * kp:2 * kp + 2, sl], KP)
            with ALP(reason="fp8 qk"):
                nc.vector.tensor_scalar_add(out=dst[:, g, half, sl], in0=ps,
                                            scalar1=qkb_sb[:, j:j + 1])

        def emit_v_chunk(t, vc):
            fw = 512 if vc == 0 else 256
            ps = ps_mm.tile([P, 512], F32, tag="mm", name="ps_v")
            dr_group(ps[:, :fw],
                     lambda kp: h1[:, 2 * kp:2 * kp + 2, t * P:(t + 1) * P],
                     lambda kp: wqkv_sb[:, 2 * kp:2 * kp + 2,
                                        2 * D + vc * 512:2 * D + vc * 512 + fw],
                     KP)
            eng = nc.scalar.copy if (2 * t + vc) % 2 else nc.vector.tensor_copy
            with ALP(reason="fp8 v"):
                eng(out=v_aug[:, t, vc * 8:vc * 8 + fw // DH, 0:DH],
                    in_=ps[:, :fw].rearrange("p (h e) -> p h e", e=DH))

        fc1w_sb = mem.tile([P, DC, HID], FP8, tag="w1")
        fc1w_v = fc1w_d[:, :].rearrange("(ko p) m -> p ko m", p=P)
        fc2w_sb = mem.tile([P, HC, D], FP8, tag="f2")
        fc2w_v = fc2w_d[:, :].rearrange("(ko p) m -> p ko m", p=P)

        x2_fm = mem.tile([P, DC, N], F32, tag="x2")
        attn_fm = mem.tile([P, DC, N], FP8, tag="at")
        x2s = mem.tile([P, DC, 512], FP8, tag="xq")
        gelu_t = mem.tile([P, HC, 512], FP8, tag="ge")
        out_fm = mem.tile([P, DC, N], F32, tag="xo")
        rdram = dram.tile([H, IC, 512], BF16, tag="rdram")

        def emit_proj_chunk(ic, mo, q0, qw):
            """token window [ic*512+q0, +qw); x2c/x2s live at [q0, q0+qw)."""
            sl = slice(ic * 512 + q0, ic * 512 + q0 + qw)
            sq = slice(q0, q0 + qw)
            ps = ps_mm.tile([P, 512], F32, tag="mm", name="ps_proj")
            dr_group(ps[:, :qw],
                     lambda kp: projw_sb[:, 2 * kp:2 * kp + 2,
                                         mo * P:(mo + 1) * P],
                     lambda kp: attn_fm[:, 2 * kp:2 * kp + 2, sl], KP)
            nc.vector.scalar_tensor_tensor(
                out=x2_fm[:, mo, sl], in0=ps[:, :qw],
                scalar=projb_sb[:, mo:mo + 1], in1=x_fm[:, mo, sl],
                op0=ADD, op1=ADD)
            with ALP(reason="fp8 stats"):
                enq = nc.gpsimd if mo % 2 else nc.vector
                enq.tensor_mul(x2s[:, mo, sq], x2_fm[:, mo, sl],
                               x2_fm[:, mo, sl])

        def emit_ln2_stats(ic, q0, qw):
            sq = slice(q0, q0 + qw)
            gsl = slice(ic * 512 + q0, ic * 512 + q0 + qw)
            murow = rows.tile([1, 512], F32, tag="row", name="mu2row")
            psr = ps_mm.tile([32, 512], F32, tag="mm", name="ps_mu2")
            dr_group(psr[:, :qw], lambda kp: wsum_sb[:, 2 * kp:2 * kp + 2, :],
                     lambda kp: attn_fm[:, 2 * kp:2 * kp + 2, gsl], KP)
            nc.vector.tensor_scalar(out=murow[:, :qw], in0=psr[0:1, :qw],
                                    scalar1=1.0 / D, scalar2=pbs_sb[0:1, :],
                                    op0=mybir.AluOpType.mult, op1=ADD)
            nc.vector.tensor_add(murow[:, :qw], murow[:, :qw], muxr[0:1, gsl])
            mu_bf = rows.tile([1, 512], BF16, tag="rowb", name="mu2bf")
            with ALP(reason="bf16 rows"):
                nc.vector.tensor_copy(out=mu_bf[:, :qw], in_=murow[:, :qw])
            s2row = rows.tile([1, 512], F32, tag="row", name="s2row")
            pss = ps_mm.tile([32, 512], F32, tag="mm", name="ps_s22")
            dr_group(pss[:, :qw], lambda kp: ones8,
                     lambda kp: x2s[:, 2 * kp:2 * kp + 2, sq], KP)
            nc.vector.tensor_scalar_mul(out=s2row[:, :qw],
                                        in0=pss[0:1, :qw], scalar1=1.0 / D)
            var = rows.tile([1, 512], F32, tag="row", name="var2")
            nc.vector.tensor_mul(var[:, :qw], murow[:, :qw], murow[:, :qw])
            nc.vector.tensor_sub(var[:, :qw], s2row[:, :qw], var[:, :qw])
            rstd2 = rows.tile([1, 512], F32, tag="row", name="rstd2")
            nc.scalar.activation(out=rstd2[:, :qw], in_=var[:, :qw],
                                 func=AF.Sqrt, bias=eps_sb[0:1, :], scale=1.0)
            nc.vector.reciprocal(out=rstd2[:, :qw], in_=rstd2[:, :qw])
            rs_bf = rows.tile([1, 512], BF16, tag="rowb", name="rs2bf")
            with ALP(reason="bf16 rows"):
                nc.vector.tensor_copy(out=rs_bf[:, :qw], in_=rstd2[:, :qw])
            mu2_bc = bcp.tile([P, 512], F32, tag="bc", name="mu2_bc")
            rstd2_bc = bcp.tile([P, 512], F32, tag="bc", name="rstd2_bc")
            psb1 = ps_mm.tile([P, 512], F32, tag="mm", name="psb_mu2")
            nc.tensor.matmul(psb1[:, :qw], onesb_r, mu_bf[:, :qw],
                             start=True, stop=True)
            nc.vector.tensor_copy(out=mu2_bc[:, :qw], in_=psb1[:, :qw])
            psb2 = ps_mm.tile([P, 512], F32, tag="mm", name="psb_rs2")
            nc.tensor.matmul(psb2[:, :qw], onesb_r, rs_bf[:, :qw],
                             start=True, stop=True)
            nc.vector.tensor_copy(out=rstd2_bc[:, :qw], in_=psb2[:, :qw])
            return mu2_bc, rstd2_bc

        h2 = [None, None]

        def emit_ln2_apply(ic, bcs, q0, qw, dcs):
            mu2_bc, rstd2_bc = bcs
            sl = slice(ic * 512 + q0, ic * 512 + q0 + qw)
            sq = slice(q0, q0 + qw)
            if h2[ic] is None:
                h2[ic] = mem.tile([P, DC, 512], FP8, tag="ha", name=f"h2_{ic}")
            for dc in dcs:
                eng = nc.gpsimd if dc >= 4 else nc.vector
                lt = ltp.tile([P, 512], BF16, tag="lt", name="ln2_tmp")
                with ALP(reason="ln2"):
                    eng.tensor_sub(lt[:, :qw], x2_fm[:, dc, sl], mu2_bc[:, :qw])
                    eng.tensor_mul(h2[ic][:, dc, sq], lt[:, :qw],
                                   rstd2_bc[:, :qw])

        def emit_fc1_chunk(ic, mo, q0=0, qw=512):
            sq = slice(q0, q0 + qw)
            ps = ps_mm.tile([P, 512], F32, tag="mm", name="ps_fc1")
            dr_group(ps[:, :qw],
                     lambda kp: fc1w_sb[:, 2 * kp:2 * kp + 2,
                                        mo * P:(mo + 1) * P],
                     lambda kp: h2[ic][:, 2 * kp:2 * kp + 2, sq], KP)
            with ALP(reason="fp8 gelu"):
                nc.scalar.activation(out=gelu_t[:, mo, sq], in_=ps[:, :qw],
                                     func=AF.Gelu,
                                     bias=fc1b_sb[:, mo:mo + 1], scale=1.0)

        def emit_fc2_chunk(ic, mo, q0, qw):
            sl = slice(ic * 512 + q0, ic * 512 + q0 + qw)
            sq = slice(q0, q0 + qw)
            ps = ps_mm.tile([P, 512], F32, tag="mm", name="ps_fc2")
            dr_group(ps[:, :qw],
                     lambda kp: fc2w_sb[:, 2 * kp:2 * kp + 2,
                                        mo * P:(mo + 1) * P],
                     lambda kp: gelu_t[:, 2 * kp:2 * kp + 2, sq], HC // 2)
            nc.vector.scalar_tensor_tensor(
                out=out_fm[:, mo, sl], in0=ps[:, :qw],
                scalar=fc2b_sb[:, mo:mo + 1], in1=x2_fm[:, mo, sl],
                op0=ADD, op1=ADD)

        def emit_exit_tr(t, tail=False):
            y_stage = mem2.tile([P, D], F32, tag="ys", name="y_stage")
            for dg in range(2):
                pt = ps_mm.tile([P, 3, P], F32, tag="mm", name="ps_tr2")
                for q in range(3):
                    dc = dg * 3 + q
                    nc.tensor.transpose(pt[:, q, :],
                                        out_fm[:, dc, t * P:(t + 1) * P],
                                        ident)
                eng = nc.scalar.copy if tail and dg % 2 \
                    else nc.vector.tensor_copy
                eng(out=y_stage[:, dg * 3 * P:(dg + 1) * 3 * P],
                    in_=pt.rearrange("p a b -> p (a b)"))
            nc.sync.dma_start(out=y_d[t * P:(t + 1) * P, :], in_=y_stage)

        # ---- attention + work-queue schedule ----
        wq = collections.deque()

        def drain(n):
            for _ in range(min(n, len(wq))):
                wq.popleft()()

        def refill(ic, h):
            if ic == 0:
                if h == 0:
                    for t in range(NT):
                        for vc in range(2):
                            wq.append(lambda t=t, vc=vc: emit_v_chunk(t, vc))
                elif h == 1:
                    for j in (2, 3, 8, 9):
                        for i2 in range(IC):
                            wq.append(lambda j=j, i2=i2: emit_qk_chunk(j, i2))
                elif h == 2:
                    for j in (4, 5, 10, 11):
                        for i2 in range(IC):
                            wq.append(lambda j=j, i2=i2: emit_qk_chunk(j, i2))
                if 7 <= h < 10:
                    for ko in (2 * (h - 7), 2 * (h - 7) + 1):
                        for c in range(2):
                            nc.sync.dma_start(
                                out=fc1w_sb[:, ko, 1536 * c:1536 * (c + 1)],
                                in_=fc1w_v[:, ko, 1536 * c:1536 * (c + 1)])
                elif h == 11:
                    for ko in range(4):
                        nc.sync.dma_start(out=fc2w_sb[:, ko, :],
                                          in_=fc2w_v[:, ko, :])
            else:
                if h < 5:
                    for ko in range(4 * h + 4, 4 * h + 8):
                        nc.sync.dma_start(out=fc2w_sb[:, ko, :],
                                          in_=fc2w_v[:, ko, :])
                if h == 1:
                    # attn_fm(ic0) complete once attnV(h11, ic0) drained (h0)
                    for mo in range(DC):
                        wq.append(lambda mo=mo: emit_proj_chunk(0, mo, 0, 512))
                elif h == 2:
                    def stats0():
                        _st["bcs0"] = emit_ln2_stats(0, 0, 512)
                    wq.append(stats0)
                    wq.append(lambda: emit_ln2_apply(0, _st["bcs0"], 0, 512,
                                                     range(3)))
                    wq.append(lambda: emit_ln2_apply(0, _st["bcs0"], 0, 512,
                                                     range(3, DC)))
                elif h in (3, 4, 5, 6, 7, 8):
                    for mo in range(4 * (h - 3), 4 * (h - 3) + 4):
                        wq.append(lambda mo=mo: emit_fc1_chunk(0, mo))
                elif h == 9:
                    for mo in range(DC):
                        wq.append(lambda mo=mo: emit_fc2_chunk(0, mo, 0, 512))
                elif h == 10:
                    for t in range(4):
                        wq.append(lambda t=t: emit_exit_tr(t))

        def emit_attnv(h, ic, expT):
            pso = ps_att.tile([VA, 512], F32, tag="att", name="pso")
            for c in range(NT // 2):
                nc.tensor.matmul(pso, v_aug[:, 2 * c:2 * c + 2, h, :],
                                 expT[:, 2 * c:2 * c + 2, :],
                                 start=(c == 0), stop=(c == NT // 2 - 1),
                                 perf_mode=DR)
            rec = recp.tile([1, 512], BF16, tag="rec", name="rec")
            with ALP(reason="bf16 recip"):
                nc.vector.reciprocal(out=rec, in_=pso[DH:DH + 1, :])
            rb = rbp.tile([DH, 512], BF16, tag="rb", name="rb")
            if h >= H - 2:
                # PE broadcast: no DMA latency right before the tail
                psb = ps_mm.tile([P, 512], F32, tag="mm", name="psb")
                nc.tensor.matmul(psb[0:DH, :], onesb_r[:, 0:DH], rec,
                                 start=True, stop=True)
                with ALP(reason="bf16 rb"):
                    nc.vector.tensor_copy(out=rb, in_=psb[0:DH, :])
            else:
                nc.sync.dma_start(out=rdram[h, ic, :], in_=rec)
                dma_bcast(rb, rdram[h, ic, :])
            with ALP(reason="fp8 attn"):
                nc.vector.tensor_mul(
                    out=attn_fm[64 * (h % 2):64 * (h % 2) + 64,
                                h // 2, ic * 512:(ic + 1) * 512],
                    in0=pso[0:DH, :], in1=rb)

        _st = {}
        DEPTH = 1  # attnV(h) emitted after scores(h+DEPTH)
        with tc.tile_pool(name="ps_sc", bufs=2, space="PSUM") as ps_sc, \
             tc.tile_pool(name="ps_att", bufs=2, space="PSUM") as ps_att, \
             tc.tile_pool(name="expp", bufs=DEPTH + 2) as expp:
            for j in (6, 7, 0, 1):
                for i2 in range(IC):
                    emit_qk_chunk(j, i2)
            pend = collections.deque()
            for ic in range(IC):
                for h in range(H):
                    refill(ic, h)
                    g, b = divmod(h, 4)
                    p0 = 32 * b
                    expT = expp.tile([P, NT, 512], FP8, tag="ex", name="expT")
                    for jp in range(NT // 2):
                        ps = ps_sc.tile([P, 2, 512], F32, tag="sc",
                                        name="ps_sc")
                        for half in range(2):
                            jc = 2 * jp + half
                            nc.tensor.matmul(
                                ps[:, half, :],
                                k_fold[p0:p0 + 32, g, :, jc * P:(jc + 1) * P],
                                q_fold[p0:p0 + 32, g, :,
                                       ic * 512:(ic + 1) * 512],
                                start=True, stop=True, perf_mode=DR,
                                tile_position=(p0, 0))
                        with ALP(reason="fp8 exp"):
                            nc.scalar.activation(
                                out=expT[:, 2 * jp:2 * jp + 2, :], in_=ps,
                                func=AF.Exp, scale=0.125)
                        drain(2 if h < 5 else 1)
                    pend.append((h, ic, expT))
                    if len(pend) > DEPTH:
                        emit_attnv(*pend.popleft())
                    drain(2)
            while pend:
                emit_attnv(*pend.popleft())
                drain(2)

            # ---- tail: ic1 MLP, stages pipelined at token-quarters ----
            drain(len(wq))
            QW = 256
            for q in range(2):
                for mo in range(DC):
                    emit_proj_chunk(1, mo, q * QW, QW)
            bcs0 = emit_ln2_stats(1, 0, QW)
            emit_ln2_apply(1, bcs0, 0, QW, range(DC))
            bcs1 = emit_ln2_stats(1, QW, QW)
            for mo in range(HC):
                emit_fc1_chunk(1, mo, 0, QW)
            emit_ln2_apply(1, bcs1, QW, QW, range(DC))
            for mo in range(HC):
                emit_fc1_chunk(1, mo, QW, QW)
            for tq in range(2):
                q0 = tq * QW
                for mo in range(DC):
                    emit_fc2_chunk(1, mo, q0, QW)
                for t in range(4 + 2 * tq, 6 + 2 * tq):
                    emit_exit_tr(t, tail=True)
            dbg = getattr(nc, "_dbg", None)
            if dbg:
                nc.sync.dma_start(out=dbg["attn"][:, :, :], in_=attn_fm)
                nc.sync.dma_start(out=dbg["x2"][:, :, :], in_=x2_fm)
                nc.sync.dma_start(out=dbg["h1"][:, :, :], in_=h1)
                nc.sync.dma_start(out=dbg["h2a"][:, :, :], in_=h2[0])
                nc.sync.dma_start(out=dbg["mux"][:, :], in_=muxr)


_NC_CACHE = {}


def _get_nc(qkb_zero=False):
    key = ("nc", qkb_zero)
    if key not in _NC_CACHE:
        _NC_CACHE[key] = build_nc(qkb_zero)
    return _NC_CACHE[key]


def _fold_perm():
    perm = []
    for g in range(3):
        for half in range(2):
            for hh in range(4):
                h = 4 * g + hh
                perm.extend(range(h * 64 + 32 * half, h * 64 + 32 * half + 32))
    return np.asarray(perm)


def _prep_inputs(x, ln1_g, ln1_b, qkv_w, qkv_b, proj_w, proj_b,
                 ln2_g, ln2_b, fc1_w, fc1_b, fc2_w, fc2_b):
    f = lambda a: np.asarray(a, np.float32)
    x = f(x)
    qkv_w, qkv_b = f(qkv_w), f(qkv_b)
    proj_w, proj_b = f(proj_w), f(proj_b)
    fc1_w, fc1_b = f(fc1_w), f(fc1_b)
    fc2_w, fc2_b = f(fc2_w), f(fc2_b)
    ln1_g, ln1_b, ln2_g, ln2_b = f(ln1_g), f(ln1_b), f(ln2_g), f(ln2_b)

    # fold LN affine into the following matmul
    qkv_w_eff = ln1_g[:, None] * qkv_w
    qkv_b_eff = qkv_b + ln1_b @ qkv_w
    fc1_w_eff = (ln2_g[:, None] * fc1_w).astype(FP8_NP)
    fc1_b_eff = (fc1_b + ln2_b @ fc1_w).astype(np.float32)

    # v bias commutes through softmax -> fold into proj bias
    vb = qkv_b_eff[2 * D:]
    proj_b_eff = (proj_b + vb @ proj_w).astype(np.float32)

    # fold permutation for q/k DoubleRow scores
    perm = _fold_perm()
    wq = qkv_w_eff[:, 0:D][:, perm]
    wk = qkv_w_eff[:, D:2 * D][:, perm]
    wv = qkv_w_eff[:, 2 * D:]
    wqkv = np.concatenate([wq, wk, wv], axis=1).astype(FP8_NP)
    qkb = np.concatenate([qkv_b_eff[0:D][perm],
                          qkv_b_eff[D:2 * D][perm]]).astype(np.float32)

    proj_w8 = proj_w.astype(FP8_NP)
    wsum8 = np.repeat(proj_w8.astype(np.float32).sum(axis=1, keepdims=True),
                      32, axis=1).astype(FP8_NP)
    pbsum = np.asarray([proj_b_eff.sum() / D], np.float32)
    shared = {
        "wqkv": wqkv, "qkb": qkb, "wsum8": wsum8, "pbsum": pbsum,
        "proj_w": proj_w8, "proj_b": proj_b_eff,
        "fc1_w": fc1_w_eff, "fc1_b": fc1_b_eff,
        "fc2_w": fc2_w.astype(FP8_NP), "fc2_b": fc2_b,
    }
    n_cores = x.shape[0]
    return [{"x": np.ascontiguousarray(x[c]), **shared} for c in range(n_cores)]


def kernel(**inputs):
    in_maps = _prep_inputs(**inputs)
    nc = _get_nc(qkb_zero=bool(np.all(in_maps[0]["qkb"] == 0.0)))
    res = run_bass_kernel_spmd(nc, in_maps, core_ids=list(range(len(in_maps))))
    return np.stack([r["y"] for r in res.results], axis=0)


if __name__ == "__main__":
    import reference
    inputs = {k: np.asarray(v) for k, v in reference.setup_inputs().items()}
    out = kernel(**inputs)
    print("kernel out", out.shape, out.dtype)


# revision 42
# speedup vs baseline: 1.4135x; 1.0029x over previous
"""Trainium2 Bass kernel for a dense transformer block (PreNorm attn + MLP).

Full inputs: x [8, 1024, 768] f32 + LN/attn/MLP weights.
Sharding: pure data-parallel — batch 8 across 8 NeuronCores, no collectives.

Per-core design (tokens n=1024, d=768, heads=12, dh=64, hidden=3072):
  - Residual spine fp32 (x2) / bf16 (x), FEATURE-major; weights [d_in, d_out]
    serve as lhsT directly.
  - All deep matmuls fp8e4 DoubleRow: one instruction contracts TWO 128-deep
    k-tiles at 0.5 cycles/moving-row.
  - Scores: q/k "folded" — head h on 32 partitions, dh split into the 2
    DoubleRow k-tiles (host-side qkv_w column permutation makes this free).
  - attnV: lhsT = v_aug [j, 2, 96]: 64 v dims + ones col (softmax
    denominator) + 31 zero pad (DoubleRow needs M % 32 == 0). The v bias
    commutes through softmax (weights sum to 1) and is folded into proj_b
    host-side.
  - exp on ACT is THE bottleneck (~100us); everything else is scheduled to
    hide under it: token-half (ic) outer / heads inner; a work queue drains
    one MLP/qkv chunk per scores-pair so the in-order PE queue never blocks
    on a pending PSUM drain.
  - LN stats feature-major: bf16 ones-matmuls (LN1) / fp8 DoubleRow
    ones-matmuls (LN2), Rsqrt on ACT, PE ones-broadcast into PSUM (LN1) or
    gpsimd partition_broadcast (LN2).
  - Tail (ic1 MLP) runs at token-quarter granularity to pipeline stages.
"""

import collections

import numpy as np
import ml_dtypes

import concourse.bass as bass
import concourse.tile as tile
from concourse import mybir
from concourse.masks import make_identity
from concourse.bass_utils import run_bass_kernel_spmd

F32 = mybir.dt.float32
BF16 = mybir.dt.bfloat16
FP8 = mybir.dt.float8e4
BF16_NP = ml_dtypes.bfloat16
FP8_NP = ml_dtypes.float8_e4m3
AF = mybir.ActivationFunctionType
DR = mybir.MatmulPerfMode.DoubleRow
ADD = mybir.AluOpType.add

N = 1024          # tokens per core
D = 768           # model dim
H = 12            # heads
DH = 64           # head dim
HID = 3072        # mlp hidden
P = 128
NT = N // P       # 8 token chunks
DC = D // P       # 6 feature chunks
HC = HID // P     # 24 hidden chunks
KP = DC // 2      # 3 contraction pairs for d=768
IC = 2            # token halves of 512
VA = 96           # attnV stationary cols: 64 v + ones + 31 pad
EPS = 1e-5


def build_nc(qkb_zero=False):
    nc = bass.Bass("TRN2")

    x_d = nc.dram_tensor("x", [N, D], BF16, kind="ExternalInput")
    wqkv_d = nc.dram_tensor("wqkv", [D, 3 * D], FP8, kind="ExternalInput")
    qkb_d = nc.dram_tensor("qkb", [2 * D], F32, kind="ExternalInput")
    projw_d = nc.dram_tensor("proj_w", [D, D], FP8, kind="ExternalInput")
    wsum_d = nc.dram_tensor("wsum8", [D, 32], FP8, kind="ExternalInput")
    pbsum_d = nc.dram_tensor("pbsum", [1], F32, kind="ExternalInput")
    projb_d = nc.dram_tensor("proj_b", [D], F32, kind="ExternalInput")
    fc1w_d = nc.dram_tensor("fc1_w", [D, HID], FP8, kind="ExternalInput")
    fc1b_d = nc.dram_tensor("fc1_b", [HID], F32, kind="ExternalInput")
    fc2w_d = nc.dram_tensor("fc2_w", [HID, D], FP8, kind="ExternalInput")
    fc2b_d = nc.dram_tensor("fc2_b", [D], F32, kind="ExternalInput")
    y_d = nc.dram_tensor("y", [N, D], F32, kind="ExternalOutput")
    nc._dbg = {
        "attn": nc.dram_tensor("dbg_attn", [P, DC, N], FP8,
                               kind="ExternalOutput"),
        "x2": nc.dram_tensor("dbg_x2", [P, DC, N], F32,
                             kind="ExternalOutput"),
        "h1": nc.dram_tensor("dbg_h1", [P, DC, N], FP8,
                             kind="ExternalOutput"),
        "h2a": nc.dram_tensor("dbg_h2a", [P, DC, 512], FP8,
                              kind="ExternalOutput"),
        "mux": nc.dram_tensor("dbg_mux", [1, N], F32,
                              kind="ExternalOutput"),
    }

    with tile.TileContext(nc) as tc:
        _body(nc, tc, x_d, wqkv_d, qkb_d, projw_d, projb_d,
              fc1w_d, fc1b_d, fc2w_d, fc2b_d, y_d, qkb_zero,
              wsum_d, pbsum_d)
    # this container's walrus accepts at most 1 sync wait per instruction
    # (2 on EventSemaphore); redistribute excess waits like Bacc.compile does
    import bass_rust as _br
    _br.move_matmul_waits_to_ldweights(nc.m)
    _br.generate_event_semaphores(nc)
    return nc


def _body(nc, tc, x_d, wqkv_d, qkb_d, projw_d, projb_d,
          fc1w_d, fc1b_d, fc2w_d, fc2b_d, y_d, qkb_zero,
          wsum_d, pbsum_d):
    from contextlib import ExitStack
    with ExitStack() as ctx:
        consts = ctx.enter_context(tc.tile_pool(name="consts", bufs=1))
        rows = ctx.enter_context(tc.tile_pool(name="rows", bufs=3))
        recp = ctx.enter_context(tc.tile_pool(name="recp", bufs=2))
        rbp = ctx.enter_context(tc.tile_pool(name="rbp", bufs=2))
        ltp = ctx.enter_context(tc.tile_pool(name="ltp", bufs=2))
        bcp = ctx.enter_context(tc.tile_pool(name="bcp", bufs=2))
        dram = ctx.enter_context(tc.tile_pool(name="dram", bufs=1, space="DRAM"))
        ps_mm = ctx.enter_context(tc.tile_pool(name="ps_mm", bufs=2, space="PSUM"))
        mem = ctx.enter_context(tc.tile_pool(name="mem", bufs=1))
        mem2 = ctx.enter_context(tc.tile_pool(name="mem2", bufs=2))

        ALP = nc.allow_low_precision

        # ---- constants & early DMAs (x first, then q/k weights) ----
        ident = consts.tile([P, P], F32, tag="ident")
        make_identity(nc, ident)
        ident_bf = consts.tile([P, P], BF16, tag="ident_bf")
        make_identity(nc, ident_bf)
        x_tok = mem.tile([P, NT, D], BF16, tag="xo")           # -> out_fm later
        x_view = x_d[:, :].rearrange("(t p) d -> p t d", p=P)
        for t in range(4):
            for c in range(2):
                nc.sync.dma_start(out=x_tok[:, t, 384 * c:384 * (c + 1)],
                                  in_=x_view[:, t, 384 * c:384 * (c + 1)])

        wqkv_sb = mem.tile([P, DC, 3 * D], FP8, tag="w1")      # -> fc1w later
        wqkv_v = wqkv_d[:, :].rearrange("(ko p) m -> p ko m", p=P)
        for k in range(DC):
            for c in range(2):
                nc.sync.dma_start(out=wqkv_sb[:, k, D * c:D * (c + 1)],
                                  in_=wqkv_v[:, k, D * c:D * (c + 1)])

        ones8 = consts.tile([P, 2, 32], FP8, tag="ones8")
        nc.vector.memset(ones8, 1.0)
        onesb_c = consts.tile([P, 1], BF16, tag="onesb_c")     # stats lhsT
        nc.vector.memset(onesb_c, 1.0)
        onesb_r = consts.tile([1, P], BF16, tag="onesb_r")     # bcast lhsT
        nc.vector.memset(onesb_r, 1.0)
        eps_sb = consts.tile([P, 1], F32, tag="eps_sb")
        nc.vector.memset(eps_sb, EPS)

        def dma_bcast(dst, dram_ap):
            kk = dst.shape[0]
            nc.sync.dma_start(out=dst, in_=bass.AP(
                tensor=dram_ap.tensor, offset=dram_ap.offset,
                ap=[[0, kk], *dram_ap.ap]))

        def load_bias_pm(dram_ap, nchunks, tag):
            t = consts.tile([P, nchunks], F32, tag=tag)
            nc.sync.dma_start(out=t, in_=dram_ap.rearrange("(mo p) -> p mo", p=P))
            return t

        qkb_sb = load_bias_pm(qkb_d[:], 2 * DC, "qkb")
        projb_sb = load_bias_pm(projb_d[:], DC, "projb")
        fc1b_sb = load_bias_pm(fc1b_d[:], HC, "fc1b")
        fc2b_sb = load_bias_pm(fc2b_d[:], DC, "fc2b")

        # deferred weight DMAs (after x-half0 / qk in the DMA queues)
        for k in range(DC):
            nc.sync.dma_start(out=wqkv_sb[:, k, 2 * D:],
                              in_=wqkv_v[:, k, 2 * D:])
        for t in range(4, NT):
            for c in range(2):
                nc.sync.dma_start(out=x_tok[:, t, 384 * c:384 * (c + 1)],
                                  in_=x_view[:, t, 384 * c:384 * (c + 1)])
        wsum_sb = consts.tile([P, DC, 32], FP8, tag="wsum")
        nc.sync.dma_start(out=wsum_sb,
                          in_=wsum_d[:, :].rearrange("(ko p) m -> p ko m", p=P))
        pbs_sb = consts.tile([1, 1], F32, tag="pbs")
        nc.sync.dma_start(out=pbs_sb, in_=pbsum_d[:])
        projw_sb = mem.tile([P, DC, D], FP8, tag="pw")
        projw_v = projw_d[:, :].rearrange("(ko p) m -> p ko m", p=P)
        for k in range(DC):
            nc.sync.dma_start(out=projw_sb[:, k, :], in_=projw_v[:, k, :])

        # v_aug: ones col + zero pad (finite garbage would still poison psum)
        v_aug = mem.tile([P, NT, H, VA], FP8, tag="vf")
        nc.vector.memset(v_aug[:, :, :, DH + 1:], 0.0)
        nc.vector.memset(v_aug[:, :, :, DH:DH + 1], 1.0)

        # ---- lead-in helpers: transposes + feature-major LN1 ----
        x_fm = mem.tile([P, DC, N], BF16, tag="xf")
        xsq = mem.tile([P, DC, 512], BF16, tag="xq")
        muxr = mem.tile([1, N], BF16, tag="mux")
        h1 = mem.tile([P, DC, N], FP8, tag="ha")               # -> h2 later
        bc_sb = [None, None]

        def lead_tr(hf, dc, pool, ptag):
            sl = slice(hf * 512, (hf + 1) * 512)
            pt = pool.tile([P, 4, P], BF16, tag=ptag)
            for q in range(4):
                t = hf * 4 + q
                nc.tensor.transpose(pt[:, q, :],
                                    x_tok[:, t, dc * P:(dc + 1) * P], ident_bf)
            eng = nc.scalar.copy if dc % 2 else nc.vector.tensor_copy
            with ALP(reason="bf16 x_fm"):
                eng(out=x_fm[:, dc, sl], in_=pt.rearrange("p a b -> p (a b)"))
            with ALP(reason="bf16 xsq"):
                eng2 = nc.gpsimd if dc % 2 else nc.vector
                eng2.tensor_mul(xsq[:, dc, :], x_fm[:, dc, sl],
                                x_fm[:, dc, sl])

        def lead_stats(hf, pool, ptag):
            sl = slice(hf * 512, (hf + 1) * 512)
            psm = pool.tile([1, 512], F32, tag=ptag, name="ps_mu1")
            for k in range(DC):
                nc.tensor.matmul(psm, onesb_c, x_fm[:, k, sl],
                                 start=(k == 0), stop=(k == DC - 1))
            murow = muxr[0:1, sl]
            with ALP(reason="bf16 mux"):
                nc.vector.tensor_scalar_mul(out=murow, in0=psm,
                                            scalar1=1.0 / D)
            pss = pool.tile([1, 512], F32, tag=ptag, name="ps_s21")
            for k in range(DC):
                nc.tensor.matmul(pss, onesb_c, xsq[:, k, :],
                                 start=(k == 0), stop=(k == DC - 1))
            var = rows.tile([1, 512], F32, tag="row", name="var1")
            nc.vector.tensor_mul(var, murow, murow)
            v2 = rows.tile([1, 512], F32, tag="row", name="v2")
            nc.vector.tensor_scalar_mul(out=v2, in0=pss, scalar1=1.0 / D)
            nc.vector.tensor_sub(var, v2, var)
            rstd = rows.tile([1, 512], F32, tag="row", name="rstd1")
            nc.scalar.activation(out=rstd, in_=var, func=AF.Sqrt,
                                 bias=eps_sb[0:1, :], scale=1.0)
            nc.vector.reciprocal(out=rstd, in_=rstd)
            mu_bf = rows.tile([1, 512], BF16, tag="rowb", name="mu1bf")
            rs_bf = rows.tile([1, 512], BF16, tag="rowb", name="rs1bf")
            with ALP(reason="bf16 rows"):
                nc.vector.tensor_copy(out=mu_bf, in_=murow)
                nc.vector.tensor_copy(out=rs_bf, in_=rstd)
            mu_ps = pool.tile([P, 512], F32, tag=ptag, name="mu1ps")
            nc.tensor.matmul(mu_ps, onesb_r, mu_bf, start=True, stop=True)
            mu_bc = bcp.tile([P, 512], BF16, tag="bc", name="mu1bc")
            with ALP(reason="bf16 bc"):
                nc.vector.tensor_copy(out=mu_bc, in_=mu_ps)
            rs_ps = pool.tile([P, 512], F32, tag=ptag, name="rs1ps")
            nc.tensor.matmul(rs_ps, onesb_r, rs_bf, start=True, stop=True)
            rs_bc = bcp.tile([P, 512], BF16, tag="bc", name="rs1bc")
            with ALP(reason="bf16 bc"):
                nc.vector.tensor_copy(out=rs_bc, in_=rs_ps)
            bc_sb[hf] = (mu_bc, rs_bc)

        def lead_ln1(hf, dcs):
            sl = slice(hf * 512, (hf + 1) * 512)
            mu_bc, rs_bc = bc_sb[hf]
            for dc in dcs:
                eng = nc.gpsimd if dc in (2, 5) else nc.vector
                lt = ltp.tile([P, 512], BF16, tag="lt", name="ln1_tmp")
                with ALP(reason="ln1"):
                    eng.tensor_sub(lt, x_fm[:, dc, sl], mu_bc)
                    eng.tensor_mul(h1[:, dc, sl], lt, rs_bc)

        # ---- lead-in inline: attention needs ALL keys, so both halves ----
        with tc.tile_pool(name="ps_lead", bufs=3, space="PSUM") as ps_lead:
            for hf in range(2):
                for dc in range(DC):
                    lead_tr(hf, dc, ps_lead, "tr")
                lead_stats(hf, ps_lead, "tr")
                lead_ln1(hf, range(DC))

        # ---- DoubleRow helpers ----
        def dr_group(ps_ap, lhs_fn, rhs_fn, nkp):
            for kp in range(nkp):
                nc.tensor.matmul(ps_ap, lhs_fn(kp), rhs_fn(kp),
                                 start=(kp == 0), stop=(kp == nkp - 1),
                                 perf_mode=DR)

        q_fold = mem.tile([P, 3, 2, N], FP8, tag="qf")
        k_fold = mem.tile([P, 3, 2, N], FP8, tag="kf")

        def emit_qk_chunk(j, ic):
            """j in 0..11: q chunks 0-5 as (g, half), k chunks 6-11."""
            g, half = divmod(j % 6, 2)
            dst = k_fold if j >= 6 else q_fold
            sl = slice(ic * 512, (ic + 1) * 512)
            ps = ps_mm.tile([P, 512], F32, tag="mm", name="ps_qk")
            dr_group(ps,
                     lambda kp: wqkv_sb[:, 2 * kp:2 * kp + 2, j * P:(j + 1) * P],
                     lambda kp: h1[:, 2 * kp:2 * kp + 2, sl], KP)
            with ALP(reason="fp8 qk"):
                nc.vector.tensor_scalar_add(out=dst[:, g, half, sl], in0=ps,
                                            scalar1=qkb_sb[:, j:j + 1])

        def emit_v_chunk(t, vc):
            fw = 512 if vc == 0 else 256
            ps = ps_mm.tile([P, 512], F32, tag="mm", name="ps_v")
            dr_group(ps[:, :fw],
                     lambda kp: h1[:, 2 * kp:2 * kp + 2, t * P:(t + 1) * P],
                     lambda kp: wqkv_sb[:, 2 * kp:2 * kp + 2,
                                        2 * D + vc * 512:2 * D + vc * 512 + fw],
                     KP)
            eng = nc.scalar.copy if (2 * t + vc) % 2 else nc.vector.tensor_copy
            with ALP(reason="fp8 v"):
                eng(out=v_aug[:, t, vc * 8:vc * 8 + fw // DH, 0:DH],
                    in_=ps[:, :fw].rearrange("p (h e) -> p h e", e=DH))

        fc1w_sb = mem.tile([P, DC, HID], FP8, tag="w1")
        fc1w_v = fc1w_d[:, :].rearrange("(ko p) m -> p ko m", p=P)
        fc2w_sb = mem.tile([P, HC, D], FP8, tag="f2")
        fc2w_v = fc2w_d[:, :].rearrange("(ko p) m -> p ko m", p=P)

        x2_fm = mem.tile([P, DC, N], F32, tag="x2")
        attn_fm = mem.tile([P, DC, N], FP8, tag="at")
        x2s = mem.tile([P, DC, 512], FP8, tag="xq")
        gelu_t = mem.tile([P, HC, 512], FP8, tag="ge")
        out_fm = mem.tile([P, DC, N], F32, tag="xo")
        rdram = dram.tile([H, IC, 512], BF16, tag="rdram")

        def emit_proj_chunk(ic, mo, q0, qw):
            """token window [ic*512+q0, +qw); x2c/x2s live at [q0, q0+qw)."""
            sl = slice(ic * 512 + q0, ic * 512 + q0 + qw)
            sq = slice(q0, q0 + qw)
            ps = ps_mm.tile([P, 512], F32, tag="mm", name="ps_proj")
            dr_group(ps[:, :qw],
                     lambda kp: projw_sb[:, 2 * kp:2 * kp + 2,
                                         mo * P:(mo + 1) * P],
                     lambda kp: attn_fm[:, 2 * kp:2 * kp + 2, sl], KP)
            nc.vector.scalar_tensor_tensor(
                out=x2_fm[:, mo, sl], in0=ps[:, :qw],
                scalar=projb_sb[:, mo:mo + 1], in1=x_fm[:, mo, sl],
                op0=ADD, op1=ADD)
            with ALP(reason="fp8 stats"):
                enq = nc.gpsimd if mo % 2 else nc.vector
                enq.tensor_mul(x2s[:, mo, sq], x2_fm[:, mo, sl],
                               x2_fm[:, mo, sl])

        def emit_ln2_stats(ic, q0, qw):
            sq = slice(q0, q0 + qw)
            gsl = slice(ic * 512 + q0, ic * 512 + q0 + qw)
            murow = rows.tile([1, 512], F32, tag="row", name="mu2row")
            psr = ps_mm.tile([32, 512], F32, tag="mm", name="ps_mu2")
            dr_group(psr[:, :qw], lambda kp: wsum_sb[:, 2 * kp:2 * kp + 2, :],
                     lambda kp: attn_fm[:, 2 * kp:2 * kp + 2, gsl], KP)
            nc.vector.tensor_scalar(out=murow[:, :qw], in0=psr[0:1, :qw],
                                    scalar1=1.0 / D, scalar2=pbs_sb[0:1, :],
                                    op0=mybir.AluOpType.mult, op1=ADD)
            nc.vector.tensor_add(murow[:, :qw], murow[:, :qw], muxr[0:1, gsl])
            mu_bf = rows.tile([1, 512], BF16, tag="rowb", name="mu2bf")
            with ALP(reason="bf16 rows"):
                nc.vector.tensor_copy(out=mu_bf[:, :qw], in_=murow[:, :qw])
            s2row = rows.tile([1, 512], F32, tag="row", name="s2row")
            pss = ps_mm.tile([32, 512], F32, tag="mm", name="ps_s22")
            dr_group(pss[:, :qw], lambda kp: ones8,
                     lambda kp: x2s[:, 2 * kp:2 * kp + 2, sq], KP)
            nc.vector.tensor_scalar_mul(out=s2row[:, :qw],
                                        in0=pss[0:1, :qw], scalar1=1.0 / D)
            var = rows.tile([1, 512], F32, tag="row", name="var2")
            nc.vector.tensor_mul(var[:, :qw], murow[:, :qw], murow[:, :qw])
            nc.vector.tensor_sub(var[:, :qw], s2row[:, :qw], var[:, :qw])
            rstd2 = rows.tile([1, 512], F32, tag="row", name="rstd2")
            nc.scalar.activation(out=rstd2[:, :qw], in_=var[:, :qw],
                                 func=AF.Sqrt, bias=eps_sb[0:1, :], scale=1.0)
            nc.vector.reciprocal(out=rstd2[:, :qw], in_=rstd2[:, :qw])
            rs_bf = rows.tile([1, 512], BF16, tag="rowb", name="rs2bf")
            with ALP(reason="bf16 rows"):
                nc.vector.tensor_copy(out=rs_bf[:, :qw], in_=rstd2[:, :qw])
            mu2_bc = bcp.tile([P, 512], F32, tag="bc", name="mu2_bc")
            rstd2_bc = bcp.tile([P, 512], F32, tag="bc", name="rstd2_bc")
            psb1 = ps_mm.tile([P, 512], F32, tag="mm", name="psb_mu2")
            nc.tensor.matmul(psb1[:, :qw], onesb_r, mu_bf[:, :qw],
                             start=True, stop=True)
            nc.vector.tensor_copy(out=mu2_bc[:, :qw], in_=psb1[:, :qw])
            psb2 = ps_mm.tile([P, 512], F32, tag="mm", name="psb_rs2")
            nc.tensor.matmul(psb2[:, :qw], onesb_r, rs_bf[:, :qw],
                             start=True, stop=True)
            nc.vector.tensor_copy(out=rstd2_bc[:, :qw], in_=psb2[:, :qw])
            return mu2_bc, rstd2_bc

        h2 = [None, None]

        def emit_ln2_apply(ic, bcs, q0, qw, dcs):
            mu2_bc, rstd2_bc = bcs
            sl = slice(ic * 512 + q0, ic * 512 + q0 + qw)
            sq = slice(q0, q0 + qw)
            if h2[ic] is None:
                h2[ic] = mem.tile([P, DC, 512], FP8, tag="ha", name=f"h2_{ic}")
            for dc in dcs:
                eng = nc.gpsimd if dc >= 4 else nc.vector
                lt = ltp.tile([P, 512], BF16, tag="lt", name="ln2_tmp")
                with ALP(reason="ln2"):
                    eng.tensor_sub(lt[:, :qw], x2_fm[:, dc, sl], mu2_bc[:, :qw])
                    eng.tensor_mul(h2[ic][:, dc, sq], lt[:, :qw],
                                   rstd2_bc[:, :qw])

        def emit_fc1_chunk(ic, mo, q0=0, qw=512):
            sq = slice(q0, q0 + qw)
            ps = ps_mm.tile([P, 512], F32, tag="mm", name="ps_fc1")
            dr_group(ps[:, :qw],
                     lambda kp: fc1w_sb[:, 2 * kp:2 * kp + 2,
                                        mo * P:(mo + 1) * P],
                     lambda kp: h2[ic][:, 2 * kp:2 * kp + 2, sq], KP)
            with ALP(reason="fp8 gelu"):
                nc.scalar.activation(out=gelu_t[:, mo, sq], in_=ps[:, :qw],
                                     func=AF.Gelu,
                                     bias=fc1b_sb[:, mo:mo + 1], scale=1.0)

        def emit_fc2_chunk(ic, mo, q0, qw):
            sl = slice(ic * 512 + q0, ic * 512 + q0 + qw)
            sq = slice(q0, q0 + qw)
            ps = ps_mm.tile([P, 512], F32, tag="mm", name="ps_fc2")
            dr_group(ps[:, :qw],
                     lambda kp: fc2w_sb[:, 2 * kp:2 * kp + 2,
                                        mo * P:(mo + 1) * P],
                     lambda kp: gelu_t[:, 2 * kp:2 * kp + 2, sq], HC // 2)
            nc.vector.scalar_tensor_tensor(
                out=out_fm[:, mo, sl], in0=ps[:, :qw],
                scalar=fc2b_sb[:, mo:mo + 1], in1=x2_fm[:, mo, sl],
                op0=ADD, op1=ADD)

        def emit_exit_tr(t, tail=False):
            y_stage = mem2.tile([P, D], F32, tag="ys", name="y_stage")
            for dg in range(2):
                pt = ps_mm.tile([P, 3, P], F32, tag="mm", name="ps_tr2")
                for q in range(3):
                    dc = dg * 3 + q
                    nc.tensor.transpose(pt[:, q, :],
                                        out_fm[:, dc, t * P:(t + 1) * P],
                                        ident)
                eng = nc.scalar.copy if tail and dg % 2 \
                    else nc.vector.tensor_copy
                eng(out=y_stage[:, dg * 3 * P:(dg + 1) * 3 * P],
                    in_=pt.rearrange("p a b -> p (a b)"))
            nc.sync.dma_start(out=y_d[t * P:(t + 1) * P, :], in_=y_stage)

        # ---- attention + work-queue schedule ----
        wq = collections.deque()

        def drain(n):
            for _ in range(min(n, len(wq))):
                wq.popleft()()

        def refill(ic, h):
            if ic == 0:
                if h == 0:
                    for t in range(NT):
                        for vc in range(2):
                            wq.append(lambda t=t, vc=vc: emit_v_chunk(t, vc))
                elif h == 1:
                    for j in (2, 3, 8, 9):
                        for i2 in range(IC):
                            wq.append(lambda j=j, i2=i2: emit_qk_chunk(j, i2))
                elif h == 2:
                    for j in (4, 5, 10, 11):
                        for i2 in range(IC):
                            wq.append(lambda j=j, i2=i2: emit_qk_chunk(j, i2))
                if 7 <= h < 10:
                    for ko in (2 * (h - 7), 2 * (h - 7) + 1):
                        for c in range(2):
                            nc.sync.dma_start(
                                out=fc1w_sb[:, ko, 1536 * c:1536 * (c + 1)],
                                in_=fc1w_v[:, ko, 1536 * c:1536 * (c + 1)])
                elif h == 11:
                    for ko in range(4):
                        nc.sync.dma_start(out=fc2w_sb[:, ko, :],
                                          in_=fc2w_v[:, ko, :])
            else:
                if h < 5:
                    for ko in range(4 * h + 4, 4 * h + 8):
                        nc.sync.dma_start(out=fc2w_sb[:, ko, :],
                                          in_=fc2w_v[:, ko, :])
                if h == 1:
                    # attn_fm(ic0) complete once attnV(h11, ic0) drained (h0)
                    for mo in range(DC):
                        wq.append(lambda mo=mo: emit_proj_chunk(0, mo, 0, 512))
                elif h == 2:
                    def stats0():
                        _st["bcs0"] = emit_ln2_stats(0, 0, 512)
                    wq.append(stats0)
                    wq.append(lambda: emit_ln2_apply(0, _st["bcs0"], 0, 512,
                                                     range(3)))
                    wq.append(lambda: emit_ln2_apply(0, _st["bcs0"], 0, 512,
                                                     range(3, DC)))
                elif h in (3, 4, 5, 6, 7, 8):
                    for mo in range(4 * (h - 3), 4 * (h - 3) + 4):
                        wq.append(lambda mo=mo: emit_fc1_chunk(0, mo))
                elif h == 9:
                    for mo in range(DC):
                        wq.append(lambda mo=mo: emit_fc2_chunk(0, mo, 0, 512))
                elif h == 10:
                    for t in range(4):
                        wq.append(lambda t=t: emit_exit_tr(t))

        def emit_attnv(h, ic, expT):
            pso = ps_att.tile([VA, 512], F32, tag="att", name="pso")
            for c in range(NT // 2):
                nc.tensor.matmul(pso, v_aug[:, 2 * c:2 * c + 2, h, :],
                                 expT[:, 2 * c:2 * c + 2, :],
                                 start=(c == 0), stop=(c == NT // 2 - 1),
                                 perf_mode=DR)
            rec = recp.tile([1, 512], BF16, tag="rec", name="rec")
            with ALP(reason="bf16 recip"):
                nc.vector.reciprocal(out=rec, in_=pso[DH:DH + 1, :])
            rb = rbp.tile([DH, 512], BF16, tag="rb", name="rb")
            if h >= H - 2:
                # PE broadcast: no DMA latency right before the tail
                psb = ps_mm.tile([P, 512], F32, tag="mm", name="psb")
                nc.tensor.matmul(psb[0:DH, :], onesb_r[:, 0:DH], rec,
                                 start=True, stop=True)
                with ALP(reason="bf16 rb"):
                    nc.vector.tensor_copy(out=rb, in_=psb[0:DH, :])
            else:
                nc.sync.dma_start(out=rdram[h, ic, :], in_=rec)
                dma_bcast(rb, rdram[h, ic, :])
            with ALP(reason="fp8 attn"):
                nc.vector.tensor_mul(
                    out=attn_fm[64 * (h % 2):64 * (h % 2) + 64,
                                h // 2, ic * 512:(ic + 1) * 512],
                    in0=pso[0:DH, :], in1=rb)

        _st = {}
        DEPTH = 1  # attnV(h) emitted after scores(h+DEPTH)
        with tc.tile_pool(name="ps_sc", bufs=2, space="PSUM") as ps_sc, \
             tc.tile_pool(name="ps_att", bufs=2, space="PSUM") as ps_att, \
             tc.tile_pool(name="expp", bufs=DEPTH + 2) as expp:
            for j in (6, 7, 0, 1):
                for i2 in range(IC):
                    emit_qk_chunk(j, i2)
            pend = collections.deque()
            for ic in range(IC):
                for h in range(H):
                    refill(ic, h)
                    g, b = divmod(h, 4)
                    p0 = 32 * b
                    expT = expp.tile([P, NT, 512], FP8, tag="ex", name="expT")
                    for jp in range(NT // 2):
                        ps = ps_sc.tile([P, 2, 512], F32, tag="sc",
                                        name="ps_sc")
                        for half in range(2):
                            jc = 2 * jp + half
                            nc.tensor.matmul(
                                ps[:, half, :],
                                k_fold[p0:p0 + 32, g, :, jc * P:(jc + 1) * P],
                                q_fold[p0:p0 + 32, g, :,
                                       ic * 512:(ic + 1) * 512],
                                start=True, stop=True, perf_mode=DR,
                                tile_position=(p0, 0))
                        with ALP(reason="fp8 exp"):
                            nc.scalar.activation(
                                out=expT[:, 2 * jp:2 * jp + 2, :], in_=ps,
                                func=AF.Exp, scale=0.125)
                        drain(2 if h < 5 else 1)
                    pend.append((h, ic, expT))
                    if len(pend) > DEPTH:
                        emit_attnv(*pend.popleft())
                    drain(2)
            while pend:
                emit_attnv(*pend.popleft())
                drain(2)

            # ---- tail: ic1 MLP, stages pipelined at token-quarters ----
            drain(len(wq))
            QW = 256
            for q in range(2):
                for mo in range(DC):
                    emit_proj_chunk(1, mo, q * QW, QW)
            bcs0 = emit_ln2_stats(1, 0, QW)
            emit_ln2_apply(1, bcs0, 0, QW, range(DC))
            bcs1 = emit_ln2_stats(1, QW, QW)
            for mo in range(HC):
                emit_fc1_chunk(1, mo, 0, QW)
            emit_ln2_apply(1, bcs1, QW, QW, range(DC))
            for mo in range(HC):
                emit_fc1_chunk(1, mo, QW, QW)
            for tq in range(2):
                q0 = tq * QW
                for mo in range(DC):
                    emit_fc2_chunk(1, mo, q0, QW)
                for t in range(4 + 2 * tq, 6 + 2 * tq):
                    emit_exit_tr(t, tail=True)
            dbg = getattr(nc, "_dbg", None)
            if dbg:
                nc.sync.dma_start(out=dbg["attn"][:, :, :], in_=attn_fm)
                nc.sync.dma_start(out=dbg["x2"][:, :, :], in_=x2_fm)
                nc.sync.dma_start(out=dbg["h1"][:, :, :], in_=h1)
                nc.sync.dma_start(out=dbg["h2a"][:, :, :], in_=h2[0])
                nc.sync.dma_start(out=dbg["mux"][:, :], in_=muxr)


_NC_CACHE = {}


def _get_nc(qkb_zero=False):
    key = ("nc", qkb_zero)
    if key not in _NC_CACHE:
        _NC_CACHE[key] = build_nc(qkb_zero)
    return _NC_CACHE[key]


def _fold_perm():
    perm = []
    for g in range(3):
        for half in range(2):
            for hh in range(4):
                h = 4 * g + hh
                perm.extend(range(h * 64 + 32 * half, h * 64 + 32 * half + 32))
    return np.asarray(perm)


def _prep_inputs(x, ln1_g, ln1_b, qkv_w, qkv_b, proj_w, proj_b,
                 ln2_g, ln2_b, fc1_w, fc1_b, fc2_w, fc2_b):
    f = lambda a: np.asarray(a, np.float32)
    x = f(x)
    qkv_w, qkv_b = f(qkv_w), f(qkv_b)
    proj_w, proj_b = f(proj_w), f(proj_b)
    fc1_w, fc1_b = f(fc1_w), f(fc1_b)
    fc2_w, fc2_b = f(fc2_w), f(fc2_b)
    ln1_g, ln1_b, ln2_g, ln2_b = f(ln1_g), f(ln1_b), f(ln2_g), f(ln2_b)

    # fold LN affine into the following matmul
    qkv_w_eff = ln1_g[:, None] * qkv_w
    qkv_b_eff = qkv_b + ln1_b @ qkv_w
    fc1_w_eff = (ln2_g[:, None] * fc1_w).astype(FP8_NP)
    fc1_b_eff = (fc1_b + ln2_b @ fc1_w).astype(np.float32)

    # v bias commutes through softmax -> fold into proj bias
    vb = qkv_b_eff[2 * D:]
    proj_b_eff = (proj_b + vb @ proj_w).astype(np.float32)

    # fold permutation for q/k DoubleRow scores
    perm = _fold_perm()
    wq = qkv_w_eff[:, 0:D][:, perm]
    wk = qkv_w_eff[:, D:2 * D][:, perm]
    wv = qkv_w_eff[:, 2 * D:]
    wqkv = np.concatenate([wq, wk, wv], axis=1).astype(FP8_NP)
    qkb = np.concatenate([qkv_b_eff[0:D][perm],
                          qkv_b_eff[D:2 * D][perm]]).astype(np.float32)

    proj_w8 = proj_w.astype(FP8_NP)
    wsum8 = np.repeat(proj_w8.astype(np.float32).sum(axis=1, keepdims=True),
                      32, axis=1).astype(FP8_NP)
    pbsum = np.asarray([proj_b_eff.sum() / D], np.float32)
    shared = {
        "wqkv": wqkv, "qkb": qkb, "wsum8": wsum8, "pbsum": pbsum,
        "proj_w": proj_w8, "proj_b": proj_b_eff,
        "fc1_w": fc1_w_eff, "fc1_b": fc1_b_eff,
        "fc2_w": fc2_w.astype(FP8_NP), "fc2_b": fc2_b,
    }
    n_cores = x.shape[0]
    return [{"x": np.ascontiguousarray(x[c]).astype(BF16_NP), **shared}
            for c in range(n_cores)]


def kernel(**inputs):
    in_maps = _prep_inputs(**inputs)
    nc = _get_nc(qkb_zero=bool(np.all(in_maps[0]["qkb"] == 0.0)))
    res = run_bass_kernel_spmd(nc, in_maps, core_ids=list(range(len(in_maps))))
    return np.stack([r["y"] for r in res.results], axis=0)


if __name__ == "__main__":
    import reference
    inputs = {k: np.asarray(v) for k, v in reference.setup_inputs().items()}
    out = kernel(**inputs)
    print("kernel out", out.shape, out.dtype)
